# revision 11
# baseline (speedup 1.0000x reference)
"""Trainium2 Bass kernel for nn_ConvertParamsTEtoParams0TE.

Computes, for B=2048, NH=128, NV=256:
  bTE2  = bTE1 + einsum("bhv,bh->bv", wtTE1, muh1) + einsum("bhv,bh->bv", wt1, muhTE1)
  wtTE2 = (0.5 * varh_diagTE1 / sqrt(varh_diag1))[:,:,None] * wt1
          + sqrt(varh_diag1)[:,:,None] * wtTE1
  sig2TE passes through unchanged.

Sharding: pure data parallel over the batch dim across 8 NeuronCores
(256 batches per core).  On-chip layout per batch: NH=128 on the
partition dim, NV=256 on the free dim.  Per batch:
  - bTE2 row: three accumulating PE matmuls into a (1,256) PSUM row
    (muh1 col as stationary vs wtTE1 tile, muhTE1 col vs wt1 tile, and a
    K=1 ones-stationary matmul that folds in the bTE1 row), then a
    ScalarE copy evacuates PSUM into a partition-0 staging tile.
  - wtTE2 tile: ScalarE per-partition-scale multiply (tmp = scale0*wt1)
    then one fused VectorE scalar_tensor_tensor (scale1*wtTE1 + tmp).
The small (B,NH) tensors are supplied host-transposed to (NH, B/8) so a
batch's h-vector is a column (a natural stationary / per-partition
scalar).  Scale factors are computed once on-device.  The two big
streams and the output stream use G-batch (2 MB) DMAs, double buffered.
"""

import os
from contextlib import ExitStack

import numpy as np

NCORES = 8
B, NH, NV = 2048, 128, 256
NB = B // NCORES  # batches per core
G = 16  # batches per streaming tile group


def build_body(ctx, tc, io, nb, g):
    import concourse.bass as bass  # noqa: F401
    from concourse import mybir

    nc = tc.nc
    f32 = mybir.dt.float32
    mult = mybir.AluOpType.mult
    add = mybir.AluOpType.add
    ng = nb // g
    assert nb % g == 0

    wtTE1, wt1, bTE1 = io["wtTE1"], io["wt1"], io["bTE1"]
    mu1T, muTE1T = io["mu1T"], io["muTE1T"]
    varh1T, varhTE1T = io["varh1T"], io["varhTE1T"]
    wtTE2, bTE2 = io["wtTE2"], io["bTE2"]

    singles = ctx.enter_context(tc.tile_pool(name="singles", bufs=1))
    in1 = ctx.enter_context(tc.tile_pool(name="in1", bufs=3))
    in2 = ctx.enter_context(tc.tile_pool(name="in2", bufs=3))
    outp = ctx.enter_context(tc.tile_pool(name="outp", bufs=2))
    binp = ctx.enter_context(tc.tile_pool(name="binp", bufs=2))
    boutp = ctx.enter_context(tc.tile_pool(name="boutp", bufs=2))
    tmps = ctx.enter_context(tc.tile_pool(name="tmps", bufs=4))
    psum = ctx.enter_context(tc.tile_pool(name="psum", bufs=8, space="PSUM"))

    f32r = mybir.dt.float32r
    # Matmul operands go through the PE in float32r (4x fp32 rate).  The
    # BIR verifier requires every producer of an fp32r-matmul operand to
    # emit float32r, so those loads/memsets write through f32r-bitcast
    # APs; elementwise consumers read the same bytes back as plain f32.

    # Whole-shard small tensors, pre-transposed to (NH, nb).
    mu1T_t = singles.tile([NH, nb], f32)
    nc.sync.dma_start(mu1T_t[:].bitcast(f32r), mu1T.bitcast(f32r))
    muTE1T_t = singles.tile([NH, nb], f32)
    nc.sync.dma_start(muTE1T_t[:].bitcast(f32r), muTE1T.bitcast(f32r))
    varh1T_t = singles.tile([NH, nb], f32)
    nc.sync.dma_start(varh1T_t[:], varh1T)
    varhTE1T_t = singles.tile([NH, nb], f32)
    nc.sync.dma_start(varhTE1T_t[:], varhTE1T)
    ones_t = singles.tile([1, 2], f32)
    nc.sync.dma_start(ones_t[:].bitcast(f32r), io["ones"].bitcast(f32r))

    # scale1 = sqrt(varh_diag1); scale0 = 0.5 * varh_diagTE1 / scale1
    scale1_t = singles.tile([NH, nb], f32)
    nc.scalar.sqrt(scale1_t[:], varh1T_t[:])
    recip_t = singles.tile([NH, nb], f32)
    nc.vector.reciprocal(recip_t[:], scale1_t[:])
    scale0_t = singles.tile([NH, nb], f32)
    nc.vector.scalar_tensor_tensor(
        scale0_t[:], varhTE1T_t[:], 0.5, recip_t[:], op0=mult, op1=mult
    )

    for gi in range(ng):
        sl = slice(gi * g, (gi + 1) * g)
        wtTE1_t = in1.tile([NH, g, NV], f32)
        wt1_t = in2.tile([NH, g, NV], f32)
        # per-batch 128KB DMAs: each reads one contiguous DRAM block, so
        # the SDMA engines see sequential 1KB pieces they can concat,
        # instead of 128KB-strided chunks from a whole-group transpose.
        for j in range(g):
            nc.sync.dma_start(
                wtTE1_t[:, j, :].bitcast(f32r), wtTE1[gi * g + j].bitcast(f32r)
            )
            nc.sync.dma_start(
                wt1_t[:, j, :].bitcast(f32r), wt1[gi * g + j].bitcast(f32r)
            )
        bTE1_t = binp.tile([1, g, NV], f32)
        nc.sync.dma_start(bTE1_t[:].bitcast(f32r), bTE1[sl].unsqueeze(0).bitcast(f32r))

        wtTE2_t = outp.tile([NH, g, NV], f32)
        # Per batch pair (j0=2jp, j1=2jp+1) the PE writes a (2, 2*NV) PSUM
        # block: row0's [0:NV] half is batch j0's bTE2, row1's [NV:2NV]
        # half is batch j1's (the other halves are don't-care cross terms,
        # and the K=1 ones-matmul adds each bTE1 row to the matching half).
        bstage_t = boutp.tile([2, g // 2, 2 * NV], f32)

        for jp in range(g // 2):
            j0 = 2 * jp
            idx0 = gi * g + j0
            prow = psum.tile([2, 2 * NV], f32)
            nc.tensor.matmul(
                prow[:],
                mu1T_t[:, idx0 : idx0 + 2].bitcast(f32r),
                wtTE1_t[:, j0 : j0 + 2, :].bitcast(f32r),
                start=True, stop=False,
            )
            nc.tensor.matmul(
                prow[:],
                muTE1T_t[:, idx0 : idx0 + 2].bitcast(f32r),
                wt1_t[:, j0 : j0 + 2, :].bitcast(f32r),
                start=False, stop=False,
            )
            nc.tensor.matmul(
                prow[:],
                ones_t[:].bitcast(f32r),
                bTE1_t[:, j0 : j0 + 2, :].bitcast(f32r),
                start=False, stop=True,
            )
            nc.scalar.copy(bstage_t[:, jp, :], prow[:])

            for j in (j0, j0 + 1):
                idx = gi * g + j
                tmp_t = tmps.tile([NH, NV], f32)
                nc.scalar.mul(
                    tmp_t[:], wt1_t[:, j, :], mul=scale0_t[:, idx : idx + 1]
                )
                nc.vector.scalar_tensor_tensor(
                    wtTE2_t[:, j, :], wtTE1_t[:, j, :],
                    scale1_t[:, idx : idx + 1], tmp_t[:], op0=mult, op1=add,
                )

        for j in range(g):
            nc.gpsimd.dma_start(wtTE2[gi * g + j], wtTE2_t[:, j, :])
        # valid halves: even batches from row0 [0:NV], odd from row1 [NV:2NV]
        nc.gpsimd.dma_start(
            bTE2[gi * g : (gi + 1) * g : 2, :].unsqueeze(0),
            bstage_t[0:1, :, 0:NV],
        )
        nc.gpsimd.dma_start(
            bTE2[gi * g + 1 : (gi + 1) * g : 2, :].unsqueeze(0),
            bstage_t[1:2, :, NV : 2 * NV],
        )


def _build(nb=NB, g=G, enable_asserts=False):
    import concourse.bacc as bacc
    import concourse.tile as tile
    from concourse import mybir

    f32 = mybir.dt.float32
    nc = bacc.Bacc(
        "TRN2",
        target_bir_lowering=False,
        debug=False,
        enable_asserts=enable_asserts,
        num_devices=NCORES,
    )
    io = {
        "wtTE1": nc.dram_tensor("wtTE1", (nb, NH, NV), f32, kind="ExternalInput").ap(),
        "wt1": nc.dram_tensor("wt1", (nb, NH, NV), f32, kind="ExternalInput").ap(),
        "bTE1": nc.dram_tensor("bTE1", (nb, NV), f32, kind="ExternalInput").ap(),
        "mu1T": nc.dram_tensor("mu1T", (NH, nb), f32, kind="ExternalInput").ap(),
        "muTE1T": nc.dram_tensor("muTE1T", (NH, nb), f32, kind="ExternalInput").ap(),
        "varh1T": nc.dram_tensor("varh1T", (NH, nb), f32, kind="ExternalInput").ap(),
        "varhTE1T": nc.dram_tensor(
            "varhTE1T", (NH, nb), f32, kind="ExternalInput"
        ).ap(),
        "ones": nc.dram_tensor("ones", (1, 2), f32, kind="ExternalInput").ap(),
        "wtTE2": nc.dram_tensor("wtTE2", (nb, NH, NV), f32, kind="ExternalOutput").ap(),
        "bTE2": nc.dram_tensor("bTE2", (nb, NV), f32, kind="ExternalOutput").ap(),
    }
    with tile.TileContext(nc) as tc:
        with ExitStack() as ctx:
            build_body(ctx, tc, io, nb, g)
    nc.compile()
    return nc


def _shard_in_maps(inputs):
    wtTE1 = np.ascontiguousarray(np.asarray(inputs["wtTE1"], dtype=np.float32))
    wt1 = np.ascontiguousarray(np.asarray(inputs["wt1"], dtype=np.float32))
    bTE1 = np.ascontiguousarray(np.asarray(inputs["bTE1"], dtype=np.float32))
    muh1 = np.asarray(inputs["muh1"], dtype=np.float32)
    muhTE1 = np.asarray(inputs["muhTE1"], dtype=np.float32)
    varh1 = np.asarray(inputs["varh_diag1"], dtype=np.float32)
    varhTE1 = np.asarray(inputs["varh_diagTE1"], dtype=np.float32)

    in_maps = []
    for c in range(NCORES):
        s = slice(c * NB, (c + 1) * NB)
        in_maps.append(
            {
                "wtTE1": wtTE1[s],
                "wt1": wt1[s],
                "bTE1": bTE1[s],
                "mu1T": np.ascontiguousarray(muh1[s].T),
                "muTE1T": np.ascontiguousarray(muhTE1[s].T),
                "varh1T": np.ascontiguousarray(varh1[s].T),
                "varhTE1T": np.ascontiguousarray(varhTE1[s].T),
                "ones": np.ones((1, 2), dtype=np.float32),
            }
        )
    return in_maps


_NC_CACHE = {}
LAST_RESULT = None  # BassKernelResults of the most recent kernel() call


def kernel(**inputs):
    global LAST_RESULT
    from concourse.bass_utils import run_bass_kernel_spmd

    if "nc" not in _NC_CACHE:
        _NC_CACHE["nc"] = _build()
    nc = _NC_CACHE["nc"]

    in_maps = _shard_in_maps(inputs)
    trace = bool(int(os.environ.get("KERNEL_TRACE", "0")))
    res = run_bass_kernel_spmd(
        nc, in_maps, core_ids=list(range(NCORES)), trace=trace
    )
    LAST_RESULT = res

    bTE2 = np.concatenate([r["bTE2"] for r in res.results], axis=0)
    wtTE2 = np.concatenate([r["wtTE2"] for r in res.results], axis=0)
    sig2TE = np.asarray(inputs["sig2TE"], dtype=np.float32)
    return (sig2TE, bTE2, wtTE2)


# revision 14
# speedup vs baseline: 1.1944x; 1.1944x over previous
"""Trainium2 Bass kernel for nn_ConvertParamsTEtoParams0TE.

Computes, for B=2048, NH=128, NV=256:
  bTE2  = bTE1 + einsum("bhv,bh->bv", wtTE1, muh1) + einsum("bhv,bh->bv", wt1, muhTE1)
  wtTE2 = (0.5 * varh_diagTE1 / sqrt(varh_diag1))[:,:,None] * wt1
          + sqrt(varh_diag1)[:,:,None] * wtTE1
  sig2TE passes through unchanged.

Sharding: pure data parallel over the batch dim across 8 NeuronCores
(256 batches per core).  On-chip layout per batch: NH=128 on the
partition dim, NV=256 on the free dim.  Per batch:
  - bTE2 row: three accumulating PE matmuls into a (1,256) PSUM row
    (muh1 col as stationary vs wtTE1 tile, muhTE1 col vs wt1 tile, and a
    K=1 ones-stationary matmul that folds in the bTE1 row), then a
    ScalarE copy evacuates PSUM into a partition-0 staging tile.
  - wtTE2 tile: ScalarE per-partition-scale multiply (tmp = scale0*wt1)
    then one fused VectorE scalar_tensor_tensor (scale1*wtTE1 + tmp).
The small (B,NH) tensors are supplied host-transposed to (NH, B/8) so a
batch's h-vector is a column (a natural stationary / per-partition
scalar).  Scale factors are computed once on-device.  The two big
streams and the output stream use G-batch (2 MB) DMAs, double buffered.
"""

import os
from contextlib import ExitStack

import numpy as np

NCORES = 8
B, NH, NV = 2048, 128, 256
NB = B // NCORES  # batches per core
G = 16  # batches per streaming tile group


def build_body(ctx, tc, io, nb, g):
    import concourse.bass as bass  # noqa: F401
    from concourse import mybir

    nc = tc.nc
    f32 = mybir.dt.float32
    mult = mybir.AluOpType.mult
    add = mybir.AluOpType.add
    ng = nb // g
    assert nb % g == 0

    wtTE1, wt1, bTE1 = io["wtTE1"], io["wt1"], io["bTE1"]
    mu1T, muTE1T = io["mu1T"], io["muTE1T"]
    varh1T, varhTE1T = io["varh1T"], io["varhTE1T"]
    wtTE2, bTE2 = io["wtTE2"], io["bTE2"]

    singles = ctx.enter_context(tc.tile_pool(name="singles", bufs=1))
    in1 = ctx.enter_context(tc.tile_pool(name="in1", bufs=3))
    in2 = ctx.enter_context(tc.tile_pool(name="in2", bufs=3))
    outp = ctx.enter_context(tc.tile_pool(name="outp", bufs=2))
    binp = ctx.enter_context(tc.tile_pool(name="binp", bufs=2))
    boutp = ctx.enter_context(tc.tile_pool(name="boutp", bufs=2))
    tmps = ctx.enter_context(tc.tile_pool(name="tmps", bufs=4))
    psum = ctx.enter_context(tc.tile_pool(name="psum", bufs=8, space="PSUM"))

    f32r = mybir.dt.float32r
    # Matmul operands go through the PE in float32r (4x fp32 rate).  The
    # BIR verifier requires every producer of an fp32r-matmul operand to
    # emit float32r, so those loads/memsets write through f32r-bitcast
    # APs; elementwise consumers read the same bytes back as plain f32.

    # Whole-shard small tensors, pre-transposed to (NH, nb).
    mu1T_t = singles.tile([NH, nb], f32)
    nc.sync.dma_start(mu1T_t[:].bitcast(f32r), mu1T.bitcast(f32r))
    muTE1T_t = singles.tile([NH, nb], f32)
    nc.sync.dma_start(muTE1T_t[:].bitcast(f32r), muTE1T.bitcast(f32r))
    varh1T_t = singles.tile([NH, nb], f32)
    nc.sync.dma_start(varh1T_t[:], varh1T)
    varhTE1T_t = singles.tile([NH, nb], f32)
    nc.sync.dma_start(varhTE1T_t[:], varhTE1T)
    ones_t = singles.tile([1, 2], f32)
    nc.sync.dma_start(ones_t[:].bitcast(f32r), io["ones"].bitcast(f32r))

    # scale1 = sqrt(varh_diag1); scale0 = 0.5 * varh_diagTE1 / scale1
    scale1_t = singles.tile([NH, nb], f32)
    nc.scalar.sqrt(scale1_t[:], varh1T_t[:])
    recip_t = singles.tile([NH, nb], f32)
    nc.vector.reciprocal(recip_t[:], scale1_t[:])
    scale0_t = singles.tile([NH, nb], f32)
    nc.vector.scalar_tensor_tensor(
        scale0_t[:], varhTE1T_t[:], 0.5, recip_t[:], op0=mult, op1=mult
    )

    for gi in range(ng):
        sl = slice(gi * g, (gi + 1) * g)
        wtTE1_t = in1.tile([NH, g, NV], f32)
        nc.sync.dma_start(
            wtTE1_t[:].bitcast(f32r), wtTE1[sl].transpose([1, 0, 2]).bitcast(f32r)
        )
        wt1_t = in2.tile([NH, g, NV], f32)
        nc.sync.dma_start(
            wt1_t[:].bitcast(f32r), wt1[sl].transpose([1, 0, 2]).bitcast(f32r)
        )
        bTE1_t = binp.tile([1, g, NV], f32)
        nc.sync.dma_start(bTE1_t[:].bitcast(f32r), bTE1[sl].unsqueeze(0).bitcast(f32r))

        wtTE2_t = outp.tile([NH, g, NV], f32)
        # Per batch pair (j0=2jp, j1=2jp+1) the PE writes a (2, 2*NV) PSUM
        # block: row0's [0:NV] half is batch j0's bTE2, row1's [NV:2NV]
        # half is batch j1's (the other halves are don't-care cross terms,
        # and the K=1 ones-matmul adds each bTE1 row to the matching half).
        bstage_t = boutp.tile([2, g // 2, 2 * NV], f32)

        for jp in range(g // 2):
            if jp == g // 4:
                # store the first half-group as soon as its STTs are done,
                # so the store stream keeps pace and the kernel tail is
                # one half-group deep instead of a full group.
                nc.gpsimd.dma_start(
                    wtTE2[gi * g : gi * g + g // 2].transpose([1, 0, 2]),
                    wtTE2_t[:, 0 : g // 2, :],
                )
            j0 = 2 * jp
            idx0 = gi * g + j0
            prow = psum.tile([2, 2 * NV], f32)
            nc.tensor.matmul(
                prow[:],
                mu1T_t[:, idx0 : idx0 + 2].bitcast(f32r),
                wtTE1_t[:, j0 : j0 + 2, :].bitcast(f32r),
                start=True, stop=False,
            )
            nc.tensor.matmul(
                prow[:],
                muTE1T_t[:, idx0 : idx0 + 2].bitcast(f32r),
                wt1_t[:, j0 : j0 + 2, :].bitcast(f32r),
                start=False, stop=False,
            )
            nc.tensor.matmul(
                prow[:],
                ones_t[:].bitcast(f32r),
                bTE1_t[:, j0 : j0 + 2, :].bitcast(f32r),
                start=False, stop=True,
            )
            nc.scalar.copy(bstage_t[:, jp, :], prow[:])

            for j in (j0, j0 + 1):
                idx = gi * g + j
                tmp_t = tmps.tile([NH, NV], f32)
                nc.scalar.mul(
                    tmp_t[:], wt1_t[:, j, :], mul=scale0_t[:, idx : idx + 1]
                )
                nc.vector.scalar_tensor_tensor(
                    wtTE2_t[:, j, :], wtTE1_t[:, j, :],
                    scale1_t[:, idx : idx + 1], tmp_t[:], op0=mult, op1=add,
                )

        nc.gpsimd.dma_start(
            wtTE2[gi * g + g // 2 : (gi + 1) * g].transpose([1, 0, 2]),
            wtTE2_t[:, g // 2 : g, :],
        )
        # valid halves: even batches from row0 [0:NV], odd from row1 [NV:2NV]
        nc.gpsimd.dma_start(
            bTE2[gi * g : (gi + 1) * g : 2, :].unsqueeze(0),
            bstage_t[0:1, :, 0:NV],
        )
        nc.gpsimd.dma_start(
            bTE2[gi * g + 1 : (gi + 1) * g : 2, :].unsqueeze(0),
            bstage_t[1:2, :, NV : 2 * NV],
        )


def _build(nb=NB, g=G, enable_asserts=False):
    import concourse.bacc as bacc
    import concourse.tile as tile
    from concourse import mybir

    f32 = mybir.dt.float32
    nc = bacc.Bacc(
        "TRN2",
        target_bir_lowering=False,
        debug=False,
        enable_asserts=enable_asserts,
        num_devices=NCORES,
    )
    io = {
        "wtTE1": nc.dram_tensor("wtTE1", (nb, NH, NV), f32, kind="ExternalInput").ap(),
        "wt1": nc.dram_tensor("wt1", (nb, NH, NV), f32, kind="ExternalInput").ap(),
        "bTE1": nc.dram_tensor("bTE1", (nb, NV), f32, kind="ExternalInput").ap(),
        "mu1T": nc.dram_tensor("mu1T", (NH, nb), f32, kind="ExternalInput").ap(),
        "muTE1T": nc.dram_tensor("muTE1T", (NH, nb), f32, kind="ExternalInput").ap(),
        "varh1T": nc.dram_tensor("varh1T", (NH, nb), f32, kind="ExternalInput").ap(),
        "varhTE1T": nc.dram_tensor(
            "varhTE1T", (NH, nb), f32, kind="ExternalInput"
        ).ap(),
        "ones": nc.dram_tensor("ones", (1, 2), f32, kind="ExternalInput").ap(),
        "wtTE2": nc.dram_tensor("wtTE2", (nb, NH, NV), f32, kind="ExternalOutput").ap(),
        "bTE2": nc.dram_tensor("bTE2", (nb, NV), f32, kind="ExternalOutput").ap(),
    }
    with tile.TileContext(nc) as tc:
        with ExitStack() as ctx:
            build_body(ctx, tc, io, nb, g)
    nc.compile()
    return nc


def _shard_in_maps(inputs):
    wtTE1 = np.ascontiguousarray(np.asarray(inputs["wtTE1"], dtype=np.float32))
    wt1 = np.ascontiguousarray(np.asarray(inputs["wt1"], dtype=np.float32))
    bTE1 = np.ascontiguousarray(np.asarray(inputs["bTE1"], dtype=np.float32))
    muh1 = np.asarray(inputs["muh1"], dtype=np.float32)
    muhTE1 = np.asarray(inputs["muhTE1"], dtype=np.float32)
    varh1 = np.asarray(inputs["varh_diag1"], dtype=np.float32)
    varhTE1 = np.asarray(inputs["varh_diagTE1"], dtype=np.float32)

    in_maps = []
    for c in range(NCORES):
        s = slice(c * NB, (c + 1) * NB)
        in_maps.append(
            {
                "wtTE1": wtTE1[s],
                "wt1": wt1[s],
                "bTE1": bTE1[s],
                "mu1T": np.ascontiguousarray(muh1[s].T),
                "muTE1T": np.ascontiguousarray(muhTE1[s].T),
                "varh1T": np.ascontiguousarray(varh1[s].T),
                "varhTE1T": np.ascontiguousarray(varhTE1[s].T),
                "ones": np.ones((1, 2), dtype=np.float32),
            }
        )
    return in_maps


_NC_CACHE = {}
LAST_RESULT = None  # BassKernelResults of the most recent kernel() call


def kernel(**inputs):
    global LAST_RESULT
    from concourse.bass_utils import run_bass_kernel_spmd

    if "nc" not in _NC_CACHE:
        _NC_CACHE["nc"] = _build()
    nc = _NC_CACHE["nc"]

    in_maps = _shard_in_maps(inputs)
    trace = bool(int(os.environ.get("KERNEL_TRACE", "0")))
    res = run_bass_kernel_spmd(
        nc, in_maps, core_ids=list(range(NCORES)), trace=trace
    )
    LAST_RESULT = res

    bTE2 = np.concatenate([r["bTE2"] for r in res.results], axis=0)
    wtTE2 = np.concatenate([r["wtTE2"] for r in res.results], axis=0)
    sig2TE = np.asarray(inputs["sig2TE"], dtype=np.float32)
    return (sig2TE, bTE2, wtTE2)


# revision 15
# speedup vs baseline: 1.2243x; 1.0250x over previous
"""Trainium2 Bass kernel for nn_ConvertParamsTEtoParams0TE.

Computes, for B=2048, NH=128, NV=256:
  bTE2  = bTE1 + einsum("bhv,bh->bv", wtTE1, muh1) + einsum("bhv,bh->bv", wt1, muhTE1)
  wtTE2 = (0.5 * varh_diagTE1 / sqrt(varh_diag1))[:,:,None] * wt1
          + sqrt(varh_diag1)[:,:,None] * wtTE1
  sig2TE passes through unchanged.

Sharding: pure data parallel over the batch dim across 8 NeuronCores
(256 batches per core).  On-chip layout per batch: NH=128 on the
partition dim, NV=256 on the free dim.  Per batch PAIR (M=2, N=512):
  - bTE2: three accumulating float32r PE matmuls into a (2,512) PSUM
    block (mu columns as stationary vs the two batches' wtTE1/wt1 tiles,
    plus a K=1 ones-matmul folding in the bTE1 rows; each row's valid
    half is its batch's bTE2), then one ScalarE copy evacuates the block
    and two strided SWDGE stores pick the valid halves.
  - wtTE2 per batch: ScalarE per-partition-scale multiply
    (tmp = scale0*wt1) then one fused VectorE scalar_tensor_tensor
    (scale1*wtTE1 + tmp).
float32r: matmul operands are bitcast to float32r (4x the fp32 PE rate);
the producing DMAs write through f32r-typed APs, which rounds the tiles
to ~tf32 precision (measured output rel err ~1.6e-4).
The small (B,NH) tensors are supplied host-transposed to (NH, B/8) so a
batch's h-vector is a column (a natural stationary / per-partition
scalar).  Scale factors are computed once on-device.  Big streams use
G=16-batch (2 MB) DMAs: loads on the sync HWDGE ring (triple-buffered),
stores on the gpsimd SWDGE queue, so descriptor generation for loads
and stores proceeds in parallel.  Measured ~333 us on hardware
(DMA-bound: ~102 MB/core at ~324 GB/s instantaneous).
"""

import os
from contextlib import ExitStack

import numpy as np

NCORES = 8
B, NH, NV = 2048, 128, 256
NB = B // NCORES  # batches per core
G = 16  # batches per streaming tile group


def build_body(ctx, tc, io, nb, g):
    import concourse.bass as bass  # noqa: F401
    from concourse import mybir

    nc = tc.nc
    f32 = mybir.dt.float32
    mult = mybir.AluOpType.mult
    add = mybir.AluOpType.add
    ng = nb // g
    assert nb % g == 0

    wtTE1, wt1, bTE1 = io["wtTE1"], io["wt1"], io["bTE1"]
    mu1T, muTE1T = io["mu1T"], io["muTE1T"]
    varh1T, varhTE1T = io["varh1T"], io["varhTE1T"]
    wtTE2, bTE2 = io["wtTE2"], io["bTE2"]

    singles = ctx.enter_context(tc.tile_pool(name="singles", bufs=1))
    in1 = ctx.enter_context(tc.tile_pool(name="in1", bufs=3))
    in2 = ctx.enter_context(tc.tile_pool(name="in2", bufs=3))
    outp = ctx.enter_context(tc.tile_pool(name="outp", bufs=2))
    binp = ctx.enter_context(tc.tile_pool(name="binp", bufs=2))
    boutp = ctx.enter_context(tc.tile_pool(name="boutp", bufs=2))
    tmps = ctx.enter_context(tc.tile_pool(name="tmps", bufs=4))
    psum = ctx.enter_context(tc.tile_pool(name="psum", bufs=8, space="PSUM"))

    f32r = mybir.dt.float32r
    # Matmul operands go through the PE in float32r (4x fp32 rate).  The
    # BIR verifier requires every producer of an fp32r-matmul operand to
    # emit float32r, so those loads/memsets write through f32r-bitcast
    # APs; elementwise consumers read the same bytes back as plain f32.

    # Whole-shard small tensors, pre-transposed to (NH, nb).
    mu1T_t = singles.tile([NH, nb], f32)
    nc.sync.dma_start(mu1T_t[:].bitcast(f32r), mu1T.bitcast(f32r))
    muTE1T_t = singles.tile([NH, nb], f32)
    nc.sync.dma_start(muTE1T_t[:].bitcast(f32r), muTE1T.bitcast(f32r))
    varh1T_t = singles.tile([NH, nb], f32)
    nc.sync.dma_start(varh1T_t[:], varh1T)
    varhTE1T_t = singles.tile([NH, nb], f32)
    nc.sync.dma_start(varhTE1T_t[:], varhTE1T)
    ones_t = singles.tile([1, 2], f32)
    nc.sync.dma_start(ones_t[:].bitcast(f32r), io["ones"].bitcast(f32r))

    # scale1 = sqrt(varh_diag1); scale0 = 0.5 * varh_diagTE1 / scale1
    scale1_t = singles.tile([NH, nb], f32)
    nc.scalar.sqrt(scale1_t[:], varh1T_t[:])
    recip_t = singles.tile([NH, nb], f32)
    nc.vector.reciprocal(recip_t[:], scale1_t[:])
    scale0_t = singles.tile([NH, nb], f32)
    nc.vector.scalar_tensor_tensor(
        scale0_t[:], varhTE1T_t[:], 0.5, recip_t[:], op0=mult, op1=mult
    )

    for gi in range(ng):
        sl = slice(gi * g, (gi + 1) * g)
        wtTE1_t = in1.tile([NH, g, NV], f32)
        nc.sync.dma_start(
            wtTE1_t[:].bitcast(f32r), wtTE1[sl].transpose([1, 0, 2]).bitcast(f32r)
        )
        wt1_t = in2.tile([NH, g, NV], f32)
        nc.sync.dma_start(
            wt1_t[:].bitcast(f32r), wt1[sl].transpose([1, 0, 2]).bitcast(f32r)
        )
        bTE1_t = binp.tile([1, g, NV], f32)
        nc.sync.dma_start(bTE1_t[:].bitcast(f32r), bTE1[sl].unsqueeze(0).bitcast(f32r))

        wtTE2_t = outp.tile([NH, g, NV], f32)
        # Per batch pair (j0=2jp, j1=2jp+1) the PE writes a (2, 2*NV) PSUM
        # block: row0's [0:NV] half is batch j0's bTE2, row1's [NV:2NV]
        # half is batch j1's (the other halves are don't-care cross terms,
        # and the K=1 ones-matmul adds each bTE1 row to the matching half).
        bstage_t = boutp.tile([2, g // 2, 2 * NV], f32)

        for jp in range(g // 2):
            j0 = 2 * jp
            idx0 = gi * g + j0
            prow = psum.tile([2, 2 * NV], f32)
            nc.tensor.matmul(
                prow[:],
                mu1T_t[:, idx0 : idx0 + 2].bitcast(f32r),
                wtTE1_t[:, j0 : j0 + 2, :].bitcast(f32r),
                start=True, stop=False,
            )
            nc.tensor.matmul(
                prow[:],
                muTE1T_t[:, idx0 : idx0 + 2].bitcast(f32r),
                wt1_t[:, j0 : j0 + 2, :].bitcast(f32r),
                start=False, stop=False,
            )
            nc.tensor.matmul(
                prow[:],
                ones_t[:].bitcast(f32r),
                bTE1_t[:, j0 : j0 + 2, :].bitcast(f32r),
                start=False, stop=True,
            )
            nc.scalar.copy(bstage_t[:, jp, :], prow[:])

            for j in (j0, j0 + 1):
                idx = gi * g + j
                tmp_t = tmps.tile([NH, NV], f32)
                nc.scalar.mul(
                    tmp_t[:], wt1_t[:, j, :], mul=scale0_t[:, idx : idx + 1]
                )
                nc.vector.scalar_tensor_tensor(
                    wtTE2_t[:, j, :], wtTE1_t[:, j, :],
                    scale1_t[:, idx : idx + 1], tmp_t[:], op0=mult, op1=add,
                )

        nc.gpsimd.dma_start(wtTE2[sl].transpose([1, 0, 2]), wtTE2_t[:])
        # valid halves: even batches from row0 [0:NV], odd from row1 [NV:2NV]
        nc.gpsimd.dma_start(
            bTE2[gi * g : (gi + 1) * g : 2, :].unsqueeze(0),
            bstage_t[0:1, :, 0:NV],
        )
        nc.gpsimd.dma_start(
            bTE2[gi * g + 1 : (gi + 1) * g : 2, :].unsqueeze(0),
            bstage_t[1:2, :, NV : 2 * NV],
        )


def _build(nb=NB, g=G, enable_asserts=False):
    import concourse.bacc as bacc
    import concourse.tile as tile
    from concourse import mybir

    f32 = mybir.dt.float32
    nc = bacc.Bacc(
        "TRN2",
        target_bir_lowering=False,
        debug=False,
        enable_asserts=enable_asserts,
        num_devices=NCORES,
    )
    io = {
        "wtTE1": nc.dram_tensor("wtTE1", (nb, NH, NV), f32, kind="ExternalInput").ap(),
        "wt1": nc.dram_tensor("wt1", (nb, NH, NV), f32, kind="ExternalInput").ap(),
        "bTE1": nc.dram_tensor("bTE1", (nb, NV), f32, kind="ExternalInput").ap(),
        "mu1T": nc.dram_tensor("mu1T", (NH, nb), f32, kind="ExternalInput").ap(),
        "muTE1T": nc.dram_tensor("muTE1T", (NH, nb), f32, kind="ExternalInput").ap(),
        "varh1T": nc.dram_tensor("varh1T", (NH, nb), f32, kind="ExternalInput").ap(),
        "varhTE1T": nc.dram_tensor(
            "varhTE1T", (NH, nb), f32, kind="ExternalInput"
        ).ap(),
        "ones": nc.dram_tensor("ones", (1, 2), f32, kind="ExternalInput").ap(),
        "wtTE2": nc.dram_tensor("wtTE2", (nb, NH, NV), f32, kind="ExternalOutput").ap(),
        "bTE2": nc.dram_tensor("bTE2", (nb, NV), f32, kind="ExternalOutput").ap(),
    }
    with tile.TileContext(nc) as tc:
        with ExitStack() as ctx:
            build_body(ctx, tc, io, nb, g)
    nc.compile()
    return nc


def _shard_in_maps(inputs):
    wtTE1 = np.ascontiguousarray(np.asarray(inputs["wtTE1"], dtype=np.float32))
    wt1 = np.ascontiguousarray(np.asarray(inputs["wt1"], dtype=np.float32))
    bTE1 = np.ascontiguousarray(np.asarray(inputs["bTE1"], dtype=np.float32))
    muh1 = np.asarray(inputs["muh1"], dtype=np.float32)
    muhTE1 = np.asarray(inputs["muhTE1"], dtype=np.float32)
    varh1 = np.asarray(inputs["varh_diag1"], dtype=np.float32)
    varhTE1 = np.asarray(inputs["varh_diagTE1"], dtype=np.float32)

    in_maps = []
    for c in range(NCORES):
        s = slice(c * NB, (c + 1) * NB)
        in_maps.append(
            {
                "wtTE1": wtTE1[s],
                "wt1": wt1[s],
                "bTE1": bTE1[s],
                "mu1T": np.ascontiguousarray(muh1[s].T),
                "muTE1T": np.ascontiguousarray(muhTE1[s].T),
                "varh1T": np.ascontiguousarray(varh1[s].T),
                "varhTE1T": np.ascontiguousarray(varhTE1[s].T),
                "ones": np.ones((1, 2), dtype=np.float32),
            }
        )
    return in_maps


_NC_CACHE = {}
LAST_RESULT = None  # BassKernelResults of the most recent kernel() call


def kernel(**inputs):
    global LAST_RESULT
    from concourse.bass_utils import run_bass_kernel_spmd

    if "nc" not in _NC_CACHE:
        _NC_CACHE["nc"] = _build()
    nc = _NC_CACHE["nc"]

    in_maps = _shard_in_maps(inputs)
    trace = bool(int(os.environ.get("KERNEL_TRACE", "0")))
    res = run_bass_kernel_spmd(
        nc, in_maps, core_ids=list(range(NCORES)), trace=trace
    )
    LAST_RESULT = res

    bTE2 = np.concatenate([r["bTE2"] for r in res.results], axis=0)
    wtTE2 = np.concatenate([r["wtTE2"] for r in res.results], axis=0)
    sig2TE = np.asarray(inputs["sig2TE"], dtype=np.float32)
    return (sig2TE, bTE2, wtTE2)
